# revision 12
# baseline (speedup 1.0000x reference)
"""Trainium2 Bass kernel: MultiHeadDepthwiseSelfAttention.

Full inputs -> data-parallel over batch across 8 NeuronCores -> full output.

Math (reference):
    q = x*wq + bq; k = x*wk + bk; v = x*wv + bv      (per-channel depthwise)
    att = softmax(q @ k^T / sqrt(F))  per head        (H=12, d=64)
    out = (att @ v) * wo + bo

Kernel strategy (per core, 2 batches):
  - Host folds the output projection into v:  veff = wv*wo, beff = bv*wo + bo.
    Then out = (att @ (x*veff + beff_aug)) / Z  where the +bo fold works because
    appending bo[c] to every V row adds bo[c]*Z[q] to the unnormalized output.
  - On chip, work in the transposed orientation S^T[k,q] so softmax's
    normalization sum (over k) is computed by the PV matmul itself via an
    extra ones-column appended per head to V ("Vhat": H*(d+1) columns).
    exp() needs no max-subtraction: logits are O(1) by construction.
  - x ships as bf16; x^T tiles come from DMA xbar transposes (no PE time).
    Q^T/K^T are per-partition scale+bias (DVE) in bf16.
  - Matmuls run in bf16 (1 col/cycle); the two heads of a pair row-pack the
    PE array (K=64 each at row groups 0/64) and run concurrently.
  - exp is split: most k-chunks on ScalarE (ACT exp -> bf16), `dve_kcs`
    chunks on VectorE via the Schraudolph trick: bf16_bits(int16(A*s + B))
    approximates exp (~3% sawtooth; numerator+denominator partially cancel).
    The exp->S(kc+2) PSUM-slot chain is the critical path; dve_kcs picks
    which links run on DVE so ACT and DVE alternate along the chain.
  - PV runs in natural orientation (pv_nat): exp(S^T) slices [128k, 128q]
    are the stationary operand (FWL), Vhat [128, 65] per head the moving
    one; out [q, 65] accumulates over kc in PSUM. The Z ones-column lands
    per-partition, so softmax finishes with a cheap reciprocal + broadcast
    TT mul -- no PE transpose-back, no O^T drain. PSUM start=True clears
    has_written BANK-wide: only the first MM per po bank sets it.
  - pv_lag=2 issues S(kc) ahead of PV(kc-2) in the PE FIFO so the
    exp-chain link cost is exp+S, not exp+PV+S.
"""

import math
import os
import sys

for _p in ("/opt/trn_rl_repo", "/root/.axon_site/_ro/trn_rl_repo"):
    if os.path.isdir(_p) and _p not in sys.path:
        sys.path.insert(0, _p)

import numpy as np
import ml_dtypes

import concourse.bacc as bacc
import concourse.mybir as mybir
from concourse.tile import TileContext
from concourse.masks import make_identity
from concourse.bass_utils import run_bass_kernel_spmd

FP32 = mybir.dt.float32
BF16 = mybir.dt.bfloat16
I16 = mybir.dt.int16
AF = mybir.ActivationFunctionType
ALU = mybir.AluOpType

P = 128
N_CORES = 8
B, N, F, H = 16, 1024, 768, 12
LOG2E = 1.4426950408889634


def build(BPC=2, N=N, F=F, H=H, reps=1, loop_reps=None, stages=4,
          ptb=4, otb=1, budget=1, dkc=1, dve_kcs=(), C16=0.0,
          xt_mode="dma", ps_sb=2, ps_ob=3, ps_nb=1, gp_vhat=False,
          drain_kcs=None, exp_split=0, gp_qk=False, pv_nat=True, fin_tt=1,
          pv_lag=1,
          dup_s=0, dup_pv=0, dup_act=0, dup_dve=0, dup_xt=0):
    d = F // H            # head dim (64)
    dO = d + 1            # V columns per head incl. ones column
    NT = N // P           # n-tiles (= k-chunks)
    CT = F // P           # channel chunks (== head pairs)
    QB = min(512, N)      # q block (moving-dim) size
    QC = N // QB          # q blocks
    TB = QB // P          # natural q-subtiles per q block
    scale = 1.0 / math.sqrt(F)
    A16 = float((1 << 7) * LOG2E * scale)
    B16 = float(127 * (1 << 7)) - float(C16)
    assert P % d == 0 and CT == H // 2

    if pv_nat:
        ps_ob = 4  # two [P, 2*dO] po tiles per phase, double-buffered
    nc = bacc.Bacc("TRN2", target_bir_lowering=False, debug=False,
                   num_devices=N_CORES)
    x = nc.declare_dram_parameter("x", [BPC, N, F], BF16, isOutput=False)
    wq = nc.declare_dram_parameter("wq", [F], FP32, isOutput=False)
    bq = nc.declare_dram_parameter("bq", [F], FP32, isOutput=False)
    wk = nc.declare_dram_parameter("wk", [F], FP32, isOutput=False)
    bk = nc.declare_dram_parameter("bk", [F], FP32, isOutput=False)
    veff = nc.declare_dram_parameter("veff", [F], BF16, isOutput=False)
    beff = nc.declare_dram_parameter("beff", [F], BF16, isOutput=False)
    out = nc.declare_dram_parameter("out", [BPC, N, F], FP32, isOutput=True)

    with TileContext(nc) as tc:
        with (
            tc.tile_pool(name="const", bufs=1) as cpool,
            tc.tile_pool(name="xp", bufs=2) as xpool,
            tc.tile_pool(name="xtp", bufs=2) as xtpool,
            tc.tile_pool(name="qp", bufs=1) as qpool,
            tc.tile_pool(name="kp", bufs=1) as kpool,
            tc.tile_pool(name="vp", bufs=2) as vpool,
            tc.tile_pool(name="op", bufs=1) as opool,
            tc.tile_pool(name="ptp", bufs=ptb) as ptpool,
            tc.tile_pool(name="otp", bufs=otb) as otpool,
            tc.tile_pool(name="rzp", bufs=2) as rzpool,
            tc.tile_pool(name="ps_s", bufs=ps_sb, space="PSUM") as ps_s,
            tc.tile_pool(name="ps_o", bufs=ps_ob, space="PSUM") as ps_o,
            tc.tile_pool(name="ps_n", bufs=ps_nb, space="PSUM") as ps_n,
        ):
            ident = cpool.tile([P, P], FP32)
            make_identity(nc, ident[:])
            ident_b = cpool.tile([P, P], BF16)
            nc.vector.tensor_copy(out=ident_b[:], in_=ident[:])
            wq_c = cpool.tile([P, CT], FP32)
            bq_c = cpool.tile([P, CT], FP32)
            wk_c = cpool.tile([P, CT], FP32)
            bk_c = cpool.tile([P, CT], FP32)
            veff_b = cpool.tile([P, F], BF16)
            beff_b = cpool.tile([P, F], BF16)

            def emit_bcasts():
                nc.sync.dma_start(out=veff_b[:],
                                  in_=veff[None, :].broadcast_to([P, F]))
                nc.sync.dma_start(out=beff_b[:],
                                  in_=beff[None, :].broadcast_to([P, F]))

            def emit_weight_loads():
                # contiguous row loads + PE transposes into per-partition cols
                specs = ((wq_c, wq), (bq_c, bq), (wk_c, wk), (bk_c, bk))
                rows = []
                for wi, (dst, src) in enumerate(specs):
                    row = cpool.tile([1, F], FP32, tag=f"wrow{wi}",
                                     name=f"wrow{wi}")
                    nc.sync.dma_start(out=row[:], in_=src[None, :])
                    rows.append(row)
                if pv_nat:
                    pw = ps_o.tile([P, 4 * CT], FP32, tag="po", name="pw")
                else:
                    pw = ps_n.tile([P, 4 * CT], FP32, tag="pn", name="pw")
                for wi, (dst, src) in enumerate(specs):
                    for c in range(CT):
                        nc.tensor.transpose(
                            pw[:, wi * CT + c:wi * CT + c + 1],
                            rows[wi][0:1, c * P:(c + 1) * P],
                            ident[0:1, 0:1])
                for wi, (dst, src) in enumerate(specs):
                    nc.vector.tensor_copy(out=dst[:],
                                          in_=pw[:, wi * CT:(wi + 1) * CT])

            def alloc_qkv(_b):
                qts = [qpool.tile([P, N], BF16, tag=f"qt{c}", name=f"qt{c}")
                       for c in range(CT)]
                kts = [kpool.tile([P, N], BF16, tag=f"kt{c}", name=f"kt{c}")
                       for c in range(CT)]
                vts = [vpool.tile([P, H * dO], BF16, tag=f"vt{i}",
                                  name=f"vt{i}") for i in range(NT)]
                return qts, kts, vts

            TG = 4  # PE-mode transposes per psum group

            def emit_chunk(xts, xtt, qts, kts, c, groups=None):
                # Q^T/K^T rows for head pair c (DVE scale+bias), x^T either
                # DMA-transposed (xtt) or PE-transposed from natural xts
                if xt_mode == "dma":
                    veng = nc.gpsimd if gp_qk else nc.vector
                    for g in (range(2) if groups is None else groups):
                        sl = slice(g * (N // 2), (g + 1) * (N // 2))
                        veng.tensor_scalar(
                            qts[c][:, sl], xtt[c][:, sl],
                            wq_c[:, c:c + 1], bq_c[:, c:c + 1],
                            op0=ALU.mult, op1=ALU.add)
                        veng.tensor_scalar(
                            kts[c][:, sl], xtt[c][:, sl],
                            wk_c[:, c:c + 1], bk_c[:, c:c + 1],
                            op0=ALU.mult, op1=ALU.add)
                    return
                for g in (range(NT // TG) if groups is None else groups):
                    pst = ps_n.tile([P, TG * P], BF16, tag="pnb", name="pst")
                    for j in range(TG):
                        i = g * TG + j
                        nc.tensor.transpose(pst[:, j * P:(j + 1) * P],
                                            xts[i][:, c * P:(c + 1) * P],
                                            ident_b[:])
                    sl = slice(g * TG * P, (g + 1) * TG * P)
                    nc.vector.tensor_scalar(qts[c][:, sl], pst[:],
                                            wq_c[:, c:c + 1], bq_c[:, c:c + 1],
                                            op0=ALU.mult, op1=ALU.add)
                    nc.vector.tensor_scalar(kts[c][:, sl], pst[:],
                                            wk_c[:, c:c + 1], bk_c[:, c:c + 1],
                                            op0=ALU.mult, op1=ALU.add)

            def emit_vhat(xts, vts, i):
                v3 = vts[i].rearrange("p (h e) -> p h e", e=dO)
                x3 = xts[i].rearrange("p (h e) -> p h e", e=d)
                w3 = veff_b.rearrange("p (h e) -> p h e", e=d)
                b3 = beff_b.rearrange("p (h e) -> p h e", e=d)
                eng = nc.gpsimd if gp_vhat else nc.vector
                for rep_ in range(1 + dup_dve):
                    nc.vector.tensor_scalar(vts[i][:, d::dO], veff_b[:, 0:H],
                                            0.0, 1.0, op0=ALU.mult,
                                            op1=ALU.add)
                    eng.tensor_mul(v3[:, :, 0:d], x3[:], w3[:])
                    eng.tensor_add(v3[:, :, 0:d], v3[:, :, 0:d], b3[:])

            def emit_program(batches):
                NB = len(batches)
                phases = [(bi, c, qc) for bi in range(NB)
                          for c in range(CT) for qc in range(QC)]
                NPH = len(phases)
                pidx = {ph: i for i, ph in enumerate(phases)}

                # per-batch tile state, allocated lazily
                xts_of, xtt_of, qkv_of, outs_of = {}, {}, {}, {}

                def get_outs(bi):
                    if bi not in outs_of:
                        outs_of[bi] = {
                            i: opool.tile([P, F], FP32, tag=f"on{i}",
                                          name=f"on{i}") for i in range(NT)}
                    return outs_of[bi]

                # work items: (earliest, deadline, fn); deadline=None -> any
                items = []

                def add_batch_items(bi):
                    xts_of[bi] = [xpool.tile([P, F], BF16, tag=f"xt{i}",
                                             name=f"xt{i}") for i in range(NT)]
                    xtt_of[bi] = [xtpool.tile([P, N], BF16, tag=f"xtt{c}",
                                              name=f"xtt{c}")
                                  for c in range(CT)]
                    qkv_of[bi] = alloc_qkv(bi)
                    xts, xtt = xts_of[bi], xtt_of[bi]
                    qts, kts, vts = qkv_of[bi]
                    first = pidx[(bi, 0, 0)]
                    if bi == 0:
                        ear_load = 0
                    else:
                        ear_load = pidx[(bi - 1, 0, 0)]

                    def xload(bi=bi, xts=xts, xtt=xtt):
                        # chunk-0 x^T first (unblocks pair-0 preproc fast)
                        if xt_mode == "dma":
                            nc.sync.dma_start_transpose(
                                xtt[0][:], x[batches[bi], :, 0:P])
                        for i in range(NT):
                            nc.sync.dma_start(
                                out=xts[i][:],
                                in_=x[batches[bi], i * P:(i + 1) * P, :])
                        if xt_mode == "dma":
                            for rep_ in range(1 + dup_xt):
                                for c in range(1, CT):
                                    nc.sync.dma_start_transpose(
                                        xtt[c][:],
                                        x[batches[bi], :, c * P:(c + 1) * P])
                    if bi == 0:
                        def first_loads():
                            emit_weight_loads()
                            emit_bcasts()
                            xload()
                            emit_chunk(xts, xtt, qts, kts, 0)
                        items.append((0, 0, first_loads))
                    else:
                        items.append((ear_load, first, xload))
                    for i in range(NT):
                        items.append((ear_load, first,
                                      lambda i=i, xts=xts, vts=vts:
                                      emit_vhat(xts, vts, i)))
                    for c in range(CT):
                        if bi == 0 and c == 0:
                            continue
                        # chunk c of batch bi: after cur batch stops reading
                        # chunk c (pair c+1 of batch bi-1), before (bi, c, 0)
                        if bi == 0:
                            ear = 0
                        elif c + 1 < CT:
                            ear = pidx[(bi - 1, c + 1, 0)]
                        else:
                            ear = pidx[(bi, 0, 0)]
                        ngr = 2 if xt_mode == "dma" else NT // TG
                        for g in range(ngr):
                            items.append((ear, pidx[(bi, c, 0)],
                                          lambda c=c, g=g, xts=xts, xtt=xtt,
                                          qts=qts, kts=kts:
                                          emit_chunk(xts, xtt, qts, kts, c,
                                                     groups=[g])))

                for bi in range(NB):
                    add_batch_items(bi)
                items.sort(key=lambda it: (it[0], it[1] if it[1] is not None
                                           else NPH))

                def flush(i, forced_deadline, budget=budget):
                    # emit all items whose deadline == forced_deadline, plus
                    # up to `budget` items whose earliest <= i
                    rest = []
                    n = 0
                    for it in items:
                        ear, dl, fn = it
                        if dl is not None and dl <= forced_deadline:
                            fn()
                        elif ear <= i and n < budget:
                            fn()
                            n += 1
                        else:
                            rest.append(it)
                    items[:] = rest

                # pipeline state
                po2_of, pts_of, pending = {}, {}, []

                def emit_s_exp(i, kc):
                    bi, c, qc = phases[i]
                    qts, kts, _ = qkv_of[bi]
                    ps = ps_s.tile([P, 2 * QB], FP32, tag="ps", name="ps")
                    # dup knobs re-emit identical ops (idempotent) to probe
                    # each engine's marginal slack
                    for rep_ in range(1 + dup_s):
                        for e in range(2):
                            nc.tensor.matmul(
                                ps[:, e * QB:(e + 1) * QB],
                                lhsT=kts[c][e * d:(e + 1) * d,
                                            kc * P:(kc + 1) * P],
                                rhs=qts[c][e * d:(e + 1) * d,
                                           qc * QB:(qc + 1) * QB],
                                start=True, stop=True)
                    if exp_split:
                        # per-head tiles: e0 exp on ACT, e1 Schraudolph on
                        # DVE -- separate tiles so the engines overlap
                        ptA = ptpool.tile([P, QB], BF16, tag="ptA",
                                          name="ptA")
                        ptB = ptpool.tile([P, QB], BF16, tag="ptB",
                                          name="ptB")
                        nc.scalar.activation(ptA[:], ps[:, 0:QB],
                                             AF.Exp, scale=scale)
                        nc.vector.tensor_scalar(ptB.bitcast(I16)[:],
                                                ps[:, QB:], A16, B16,
                                                op0=ALU.mult, op1=ALU.add)
                        pts_of[(i, kc)] = (ptA, ptB)
                        return
                    pt = ptpool.tile([P, 2 * QB], BF16, tag="pt", name="pt")
                    if kc in dve_kcs:
                        nc.vector.tensor_scalar(pt.bitcast(I16)[:], ps[:],
                                                A16, B16, op0=ALU.mult,
                                                op1=ALU.add)
                    else:
                        for rep_ in range(1 + dup_act):
                            nc.scalar.activation(pt[:], ps[:], AF.Exp,
                                                 scale=scale)
                    pts_of[(i, kc)] = pt

                def emit_pv(i, kc):
                    if stages < 3:
                        return
                    bi, c, qc = phases[i]
                    _, _, vts = qkv_of[bi]
                    h0 = 2 * c
                    if pv_nat:
                        # natural orientation: att slice is the stationary
                        # operand, Vhat [128, dO] per head is the moving one.
                        # out[q, c] accumulates over kc; Z lands in col d of
                        # each head's dO block (per-partition -> cheap DVE
                        # normalize, no PE transpose-back).
                        if i not in po2_of:
                            po2_of[i] = [
                                ps_o.tile([P, 4 * dO], FP32, tag="po",
                                          name=f"po{j}") for j in range(2)]
                        pt = pts_of.pop((i, kc))
                        for rep_ in range(1 + (dup_pv if kc == 0 else 0)):
                            for t in range(TB):
                                po = po2_of[i][t // 2]
                                off = (t % 2) * 2 * dO
                                for e in range(2):
                                    if isinstance(pt, tuple):
                                        lhsT = pt[e][:, t * P:(t + 1) * P]
                                    else:
                                        lhsT = pt[:, e * QB + t * P:
                                                  e * QB + (t + 1) * P]
                                    # start=True clears has_written for the
                                    # WHOLE bank: only the tile's first MM
                                    # may set it; the other groups' kc==0
                                    # MMs land on cleared bits (overwrite
                                    # mode) in PE program order.
                                    nc.tensor.matmul(
                                        po[:, off + e * dO:
                                           off + (e + 1) * dO],
                                        lhsT=lhsT,
                                        rhs=vts[kc][:, (h0 + e) * dO:
                                                    (h0 + e + 1) * dO],
                                        start=(kc == 0 and t % 2 == 0
                                               and e == 0),
                                        stop=(kc == NT - 1),
                                        skip_group_check=True)
                        return
                    if i not in po2_of:
                        po2_of[i] = [ps_o.tile([dO, QB], FP32, tag="po",
                                               name=f"po{e}") for e in range(2)]
                    pt = pts_of.pop((i, kc))
                    for rep_ in range(1 + (dup_pv if kc == 0 else 0)):
                        for e in range(2):
                            rhs = (pt[e][:, 0:QB] if isinstance(pt, tuple)
                                   else pt[:, e * QB:(e + 1) * QB])
                            nc.tensor.matmul(
                                po2_of[i][e][:],
                                lhsT=vts[kc][:,
                                             (h0 + e) * dO:(h0 + e + 1) * dO],
                                rhs=rhs,
                                start=(kc == 0), stop=(kc == NT - 1))

                def emit_drain(i):
                    if stages < 4 or stages < 3:
                        return
                    bi, c, qc = phases[i]
                    last_pair = (c == CT - 1)
                    h0 = 2 * c
                    outs = get_outs(bi)
                    po2 = po2_of.pop(i)
                    if pv_nat:
                        # po tile j holds subtiles t=2j,2j+1 as
                        # [e0(64) Z0 e1(64) Z1] x2; Z cols at 64+65k.
                        def finish_nat(j, po=None, bi=bi, qc=qc, h0=h0,
                                       last_pair=last_pair):
                            po = po2[j]
                            rz = rzpool.tile([P, 4], FP32, tag="rz",
                                             name="rz")
                            nc.vector.reciprocal(rz[:], po[:, d::dO])
                            for m in range(2):
                                t = 2 * j + m
                                qsub = qc * TB + t
                                dst = outs[qsub][:, h0 * d:(h0 + 2) * d]
                                if fin_tt:
                                    src = po[:, m * 2 * dO:(m + 1) * 2 * dO]
                                    s3 = src.rearrange("p (k x) -> p k x",
                                                       x=dO)
                                    r3 = rz[:, 2 * m:2 * m + 2]
                                    r3 = r3.rearrange("p (k x) -> p k x",
                                                      x=1)
                                    d3 = dst.rearrange("p (k x) -> p k x",
                                                      x=d)
                                    nc.vector.tensor_mul(
                                        d3[:], s3[:, :, 0:d],
                                        r3.broadcast_to([P, 2, d]))
                                else:
                                    for e in range(2):
                                        nc.vector.tensor_scalar_mul(
                                            outs[qsub][:, (h0 + e) * d:
                                                       (h0 + e + 1) * d],
                                            po[:, m * 2 * dO + e * dO:
                                               m * 2 * dO + e * dO + d],
                                            rz[:, 2 * m + e:2 * m + e + 1])
                                if last_pair:
                                    nc.sync.dma_start(
                                        out=out[batches[bi],
                                                qsub * P:(qsub + 1) * P, :],
                                        in_=outs[qsub][:])
                        pending.clear()
                        pending.append(lambda: finish_nat(0))
                        pending.append(lambda: finish_nat(1))
                        return
                    ots = []
                    for e in range(2):
                        ot = otpool.tile([dO, QB], FP32, tag=f"ot{e}",
                                         name=f"ot{e}")
                        nc.vector.tensor_copy(out=ot[:], in_=po2[e][:])
                        ots.append(ot)

                    def finish(e):
                        pn = ps_n.tile([P, TB * dO], FP32, tag="pn",
                                       name="pn")
                        for t in range(TB):
                            nc.tensor.transpose(
                                pn[:, t * dO:(t + 1) * dO],
                                ots[e][:, t * P:(t + 1) * P],
                                ident[0:dO, 0:dO])
                        rz = rzpool.tile([P, TB], FP32, tag="rz", name="rz")
                        nc.vector.reciprocal(rz[:], pn[:, d::dO])
                        for t in range(TB):
                            qsub = qc * TB + t
                            nc.vector.tensor_scalar_mul(
                                outs[qsub][:, (h0 + e) * d:(h0 + e + 1) * d],
                                pn[:, t * dO:t * dO + d],
                                rz[:, t:t + 1])
                        if last_pair and e == 1:
                            for t in range(TB):
                                qsub = qc * TB + t
                                nc.sync.dma_start(
                                    out=out[batches[bi],
                                            qsub * P:(qsub + 1) * P, :],
                                    in_=outs[qsub][:])
                    pending.clear()
                    pending.append(lambda: finish(0))
                    pending.append(lambda: finish(1))

                def flush_pending(all_=False):
                    while pending:
                        pending.pop(0)()
                        if not all_:
                            return

                # prologue: phase 0 prefetch
                flush(0, 0, budget=0)
                emit_s_exp(0, 0)
                for i in range(NPH):
                    bi, c, qc = phases[i]
                    dks = drain_kcs or (dkc, dkc + 3)
                    # pv_lag=2 keeps S(kc) AHEAD of PV(kc-2) in the PE FIFO:
                    # both wait on the same exp, but S is on the exp->ps-slot
                    # critical chain while PV is not, so issue S first.
                    for kc in range(1, NT):
                        emit_s_exp(i, kc)
                        if kc in dks:
                            flush_pending()
                        if kc - pv_lag >= 0 and kc < NT - 1:
                            emit_pv(i, kc - pv_lag)
                    if i + 1 < NPH:
                        flush(i, i + 1)
                        emit_s_exp(i + 1, 0)
                    for kc in range(NT - 1 - pv_lag, NT):
                        if kc >= 0:
                            emit_pv(i, kc)
                    emit_drain(i)
                    # end of batch: flush the last drain, drop out-tile refs
                    if stages >= 4 and (i + 1 == NPH or phases[i + 1][0] != bi):
                        flush_pending(all_=True)
                        outs_of.pop(bi)
                flush(NPH, NPH, budget=99)

            if loop_reps is None:
                emit_program([bb for _ in range(reps) for bb in range(BPC)])
            else:
                with tc.For_i(0, loop_reps, 1):
                    emit_program(list(range(BPC)))
    nc.compile()
    return nc


_built = {}

# chosen production config (see sweep logs): PV-natural, exp split 5 ACT /
# 3 DVE-Schraudolph chunks, S-before-PV queue order
BEST = dict(dve_kcs=(1, 4, 6), pv_lag=2)


def _get_nc(BPC):
    if BPC not in _built:
        _built[BPC] = build(BPC=BPC, **BEST)
    return _built[BPC]


def prep_inputs(x, wq, bq, wk, bk, wv, bv, wo, bo):
    x = np.ascontiguousarray(np.asarray(x, dtype=np.float32))
    wq, bq, wk, bk, wv, bv, wo, bo = (
        np.asarray(t, dtype=np.float32)
        for t in (wq, bq, wk, bk, wv, bv, wo, bo))
    xb = x.astype(ml_dtypes.bfloat16)
    veff = (wv * wo).astype(ml_dtypes.bfloat16)
    beff = (bv * wo + bo).astype(ml_dtypes.bfloat16)
    return xb, wq, bq, wk, bk, veff, beff


def kernel(x, wq, bq, wk, bk, wv, bv, wo, bo):
    xb, wq, bq, wk, bk, veff, beff = prep_inputs(
        x, wq, bq, wk, bk, wv, bv, wo, bo)
    Bx = xb.shape[0]
    BPC = Bx // N_CORES
    assert BPC * N_CORES == Bx, (Bx, N_CORES)
    nc = _get_nc(BPC)
    in_maps = []
    for i in range(N_CORES):
        in_maps.append({
            "x": xb[i * BPC:(i + 1) * BPC],
            "wq": wq, "bq": bq, "wk": wk, "bk": bk,
            "veff": veff, "beff": beff,
        })
    res = run_bass_kernel_spmd(nc, in_maps, list(range(N_CORES)))
    return np.concatenate([r["out"] for r in res.results], axis=0)


if __name__ == "__main__":
    rng = np.random.default_rng(1)
    inputs = {
        "x": rng.standard_normal((B, N, F), dtype=np.float32),
        "wq": rng.standard_normal((F,), dtype=np.float32),
        "bq": np.zeros(F, np.float32),
        "wk": rng.standard_normal((F,), dtype=np.float32),
        "bk": np.zeros(F, np.float32),
        "wv": rng.standard_normal((F,), dtype=np.float32),
        "bv": np.zeros(F, np.float32),
        "wo": rng.standard_normal((F,), dtype=np.float32),
        "bo": np.zeros(F, np.float32),
    }
    o = kernel(**inputs)
    print("out", o.shape, o.dtype)



# revision 13
# speedup vs baseline: 1.0097x; 1.0097x over previous
"""Trainium2 Bass kernel: MultiHeadDepthwiseSelfAttention.

Full inputs -> data-parallel over batch across 8 NeuronCores -> full output.

Math (reference):
    q = x*wq + bq; k = x*wk + bk; v = x*wv + bv      (per-channel depthwise)
    att = softmax(q @ k^T / sqrt(F))  per head        (H=12, d=64)
    out = (att @ v) * wo + bo

Kernel strategy (per core, 2 batches):
  - Host folds the output projection into v:  veff = wv*wo, beff = bv*wo + bo.
    Then out = (att @ (x*veff + beff_aug)) / Z  where the +bo fold works because
    appending bo[c] to every V row adds bo[c]*Z[q] to the unnormalized output.
  - On chip, work in the transposed orientation S^T[k,q] so softmax's
    normalization sum (over k) is computed by the PV matmul itself via an
    extra ones-column appended per head to V ("Vhat": H*(d+1) columns).
    exp() needs no max-subtraction: logits are O(1) by construction.
  - x ships as bf16; x^T tiles come from DMA xbar transposes (no PE time).
    Q^T/K^T are per-partition scale+bias (DVE) in bf16.
  - Matmuls run in bf16 (1 col/cycle); the two heads of a pair row-pack the
    PE array (K=64 each at row groups 0/64) and run concurrently.
  - exp is split: most k-chunks on ScalarE (ACT exp -> bf16), `dve_kcs`
    chunks on VectorE via the Schraudolph trick: bf16_bits(int16(A*s + B))
    approximates exp (~3% sawtooth; numerator+denominator partially cancel).
    The exp->S(kc+2) PSUM-slot chain is the critical path; dve_kcs picks
    which links run on DVE so ACT and DVE alternate along the chain.
  - PV runs in natural orientation (pv_nat): exp(S^T) slices [128k, 128q]
    are the stationary operand (FWL), Vhat [128, 65] per head the moving
    one; out [q, 65] accumulates over kc in PSUM. The Z ones-column lands
    per-partition, so softmax finishes with a cheap reciprocal + broadcast
    TT mul -- no PE transpose-back, no O^T drain. PSUM start=True clears
    has_written BANK-wide: only the first MM per po bank sets it.
  - pv_lag=2 issues S(kc) ahead of PV(kc-2) in the PE FIFO so the
    exp-chain link cost is exp+S, not exp+PV+S.
"""

import math
import os
import sys

for _p in ("/opt/trn_rl_repo", "/root/.axon_site/_ro/trn_rl_repo"):
    if os.path.isdir(_p) and _p not in sys.path:
        sys.path.insert(0, _p)

import numpy as np
import ml_dtypes

import concourse.bacc as bacc
import concourse.mybir as mybir
from concourse.tile import TileContext
from concourse.masks import make_identity
from concourse.bass_utils import run_bass_kernel_spmd

FP32 = mybir.dt.float32
BF16 = mybir.dt.bfloat16
I16 = mybir.dt.int16
AF = mybir.ActivationFunctionType
ALU = mybir.AluOpType

P = 128
N_CORES = 8
B, N, F, H = 16, 1024, 768, 12
LOG2E = 1.4426950408889634


def build(BPC=2, N=N, F=F, H=H, reps=1, loop_reps=None, stages=4,
          ptb=4, otb=1, budget=1, dkc=1, dve_kcs=(), C16=0.0,
          xt_mode="dma", ps_sb=2, ps_ob=3, ps_nb=1, gp_vhat=False,
          drain_kcs=None, exp_split=0, gp_qk=False, pv_nat=True, fin_tt=1,
          pv_lag=1,
          dup_s=0, dup_pv=0, dup_act=0, dup_dve=0, dup_xt=0):
    d = F // H            # head dim (64)
    dO = d + 1            # V columns per head incl. ones column
    NT = N // P           # n-tiles (= k-chunks)
    CT = F // P           # channel chunks (== head pairs)
    QB = min(512, N)      # q block (moving-dim) size
    QC = N // QB          # q blocks
    TB = QB // P          # natural q-subtiles per q block
    scale = 1.0 / math.sqrt(F)
    A16 = float((1 << 7) * LOG2E * scale)
    B16 = float(127 * (1 << 7)) - float(C16)
    assert P % d == 0 and CT == H // 2

    if pv_nat:
        ps_ob = 4  # two [P, 2*dO] po tiles per phase, double-buffered
    nc = bacc.Bacc("TRN2", target_bir_lowering=False, debug=False,
                   num_devices=N_CORES)
    x = nc.declare_dram_parameter("x", [BPC, N, F], BF16, isOutput=False)
    wq = nc.declare_dram_parameter("wq", [F], FP32, isOutput=False)
    bq = nc.declare_dram_parameter("bq", [F], FP32, isOutput=False)
    wk = nc.declare_dram_parameter("wk", [F], FP32, isOutput=False)
    bk = nc.declare_dram_parameter("bk", [F], FP32, isOutput=False)
    veff = nc.declare_dram_parameter("veff", [F], BF16, isOutput=False)
    beff = nc.declare_dram_parameter("beff", [F], BF16, isOutput=False)
    out = nc.declare_dram_parameter("out", [BPC, N, F], FP32, isOutput=True)

    with TileContext(nc) as tc:
        with (
            tc.tile_pool(name="const", bufs=1) as cpool,
            tc.tile_pool(name="xp", bufs=2) as xpool,
            tc.tile_pool(name="xtp", bufs=2) as xtpool,
            tc.tile_pool(name="qp", bufs=1) as qpool,
            tc.tile_pool(name="kp", bufs=1) as kpool,
            tc.tile_pool(name="vp", bufs=2) as vpool,
            tc.tile_pool(name="op", bufs=1) as opool,
            tc.tile_pool(name="ptp", bufs=ptb) as ptpool,
            tc.tile_pool(name="otp", bufs=otb) as otpool,
            tc.tile_pool(name="rzp", bufs=2) as rzpool,
            tc.tile_pool(name="ps_s", bufs=ps_sb, space="PSUM") as ps_s,
            tc.tile_pool(name="ps_o", bufs=ps_ob, space="PSUM") as ps_o,
            tc.tile_pool(name="ps_n", bufs=ps_nb, space="PSUM") as ps_n,
        ):
            ident = cpool.tile([P, P], FP32)
            make_identity(nc, ident[:])
            ident_b = cpool.tile([P, P], BF16)
            nc.vector.tensor_copy(out=ident_b[:], in_=ident[:])
            wq_c = cpool.tile([P, CT], FP32)
            bq_c = cpool.tile([P, CT], FP32)
            wk_c = cpool.tile([P, CT], FP32)
            bk_c = cpool.tile([P, CT], FP32)
            veff_b = cpool.tile([P, F], BF16)
            beff_b = cpool.tile([P, F], BF16)

            def emit_bcasts():
                nc.sync.dma_start(out=veff_b[:],
                                  in_=veff[None, :].broadcast_to([P, F]))
                nc.sync.dma_start(out=beff_b[:],
                                  in_=beff[None, :].broadcast_to([P, F]))

            def emit_weight_loads():
                # contiguous row loads + PE transposes into per-partition cols
                specs = ((wq_c, wq), (bq_c, bq), (wk_c, wk), (bk_c, bk))
                rows = []
                for wi, (dst, src) in enumerate(specs):
                    row = cpool.tile([1, F], FP32, tag=f"wrow{wi}",
                                     name=f"wrow{wi}")
                    nc.sync.dma_start(out=row[:], in_=src[None, :])
                    rows.append(row)
                if pv_nat:
                    pw = ps_o.tile([P, 4 * CT], FP32, tag="po", name="pw")
                else:
                    pw = ps_n.tile([P, 4 * CT], FP32, tag="pn", name="pw")
                for wi, (dst, src) in enumerate(specs):
                    for c in range(CT):
                        nc.tensor.transpose(
                            pw[:, wi * CT + c:wi * CT + c + 1],
                            rows[wi][0:1, c * P:(c + 1) * P],
                            ident[0:1, 0:1])
                for wi, (dst, src) in enumerate(specs):
                    nc.vector.tensor_copy(out=dst[:],
                                          in_=pw[:, wi * CT:(wi + 1) * CT])

            def alloc_qkv(_b):
                qts = [qpool.tile([P, N], BF16, tag=f"qt{c}", name=f"qt{c}")
                       for c in range(CT)]
                kts = [kpool.tile([P, N], BF16, tag=f"kt{c}", name=f"kt{c}")
                       for c in range(CT)]
                vts = [vpool.tile([P, H * dO], BF16, tag=f"vt{i}",
                                  name=f"vt{i}") for i in range(NT)]
                return qts, kts, vts

            TG = 4  # PE-mode transposes per psum group

            def emit_chunk(xts, xtt, qts, kts, c, groups=None):
                # Q^T/K^T rows for head pair c (DVE scale+bias), x^T either
                # DMA-transposed (xtt) or PE-transposed from natural xts
                if xt_mode == "dma":
                    veng = nc.gpsimd if gp_qk else nc.vector
                    for g in (range(2) if groups is None else groups):
                        sl = slice(g * (N // 2), (g + 1) * (N // 2))
                        veng.tensor_scalar(
                            qts[c][:, sl], xtt[c][:, sl],
                            wq_c[:, c:c + 1], bq_c[:, c:c + 1],
                            op0=ALU.mult, op1=ALU.add)
                        veng.tensor_scalar(
                            kts[c][:, sl], xtt[c][:, sl],
                            wk_c[:, c:c + 1], bk_c[:, c:c + 1],
                            op0=ALU.mult, op1=ALU.add)
                    return
                for g in (range(NT // TG) if groups is None else groups):
                    pst = ps_n.tile([P, TG * P], BF16, tag="pnb", name="pst")
                    for j in range(TG):
                        i = g * TG + j
                        nc.tensor.transpose(pst[:, j * P:(j + 1) * P],
                                            xts[i][:, c * P:(c + 1) * P],
                                            ident_b[:])
                    sl = slice(g * TG * P, (g + 1) * TG * P)
                    nc.vector.tensor_scalar(qts[c][:, sl], pst[:],
                                            wq_c[:, c:c + 1], bq_c[:, c:c + 1],
                                            op0=ALU.mult, op1=ALU.add)
                    nc.vector.tensor_scalar(kts[c][:, sl], pst[:],
                                            wk_c[:, c:c + 1], bk_c[:, c:c + 1],
                                            op0=ALU.mult, op1=ALU.add)

            def emit_vhat(xts, vts, i):
                v3 = vts[i].rearrange("p (h e) -> p h e", e=dO)
                x3 = xts[i].rearrange("p (h e) -> p h e", e=d)
                w3 = veff_b.rearrange("p (h e) -> p h e", e=d)
                b3 = beff_b.rearrange("p (h e) -> p h e", e=d)
                eng = nc.gpsimd if gp_vhat else nc.vector
                for rep_ in range(1 + dup_dve):
                    nc.vector.tensor_scalar(vts[i][:, d::dO], veff_b[:, 0:H],
                                            0.0, 1.0, op0=ALU.mult,
                                            op1=ALU.add)
                    eng.tensor_mul(v3[:, :, 0:d], x3[:], w3[:])
                    eng.tensor_add(v3[:, :, 0:d], v3[:, :, 0:d], b3[:])

            def emit_program(batches):
                NB = len(batches)
                phases = [(bi, c, qc) for bi in range(NB)
                          for c in range(CT) for qc in range(QC)]
                NPH = len(phases)
                pidx = {ph: i for i, ph in enumerate(phases)}

                # per-batch tile state, allocated lazily
                xts_of, xtt_of, qkv_of, outs_of = {}, {}, {}, {}

                def get_outs(bi):
                    if bi not in outs_of:
                        outs_of[bi] = {
                            i: opool.tile([P, F], FP32, tag=f"on{i}",
                                          name=f"on{i}") for i in range(NT)}
                    return outs_of[bi]

                # work items: (earliest, deadline, fn); deadline=None -> any
                items = []

                def add_batch_items(bi):
                    xts_of[bi] = [xpool.tile([P, F], BF16, tag=f"xt{i}",
                                             name=f"xt{i}") for i in range(NT)]
                    xtt_of[bi] = [xtpool.tile([P, N], BF16, tag=f"xtt{c}",
                                              name=f"xtt{c}")
                                  for c in range(CT)]
                    qkv_of[bi] = alloc_qkv(bi)
                    xts, xtt = xts_of[bi], xtt_of[bi]
                    qts, kts, vts = qkv_of[bi]
                    first = pidx[(bi, 0, 0)]
                    if bi == 0:
                        ear_load = 0
                    else:
                        ear_load = pidx[(bi - 1, 0, 0)]

                    def xload(bi=bi, xts=xts, xtt=xtt):
                        # chunk-0 x^T first (unblocks pair-0 preproc fast)
                        if xt_mode == "dma":
                            nc.sync.dma_start_transpose(
                                xtt[0][:], x[batches[bi], :, 0:P])
                        for i in range(NT):
                            nc.sync.dma_start(
                                out=xts[i][:],
                                in_=x[batches[bi], i * P:(i + 1) * P, :])
                        if xt_mode == "dma":
                            for rep_ in range(1 + dup_xt):
                                for c in range(1, CT):
                                    nc.sync.dma_start_transpose(
                                        xtt[c][:],
                                        x[batches[bi], :, c * P:(c + 1) * P])
                    if bi == 0:
                        def first_loads():
                            emit_weight_loads()
                            emit_bcasts()
                            xload()
                            emit_chunk(xts, xtt, qts, kts, 0)
                        items.append((0, 0, first_loads))
                    else:
                        items.append((ear_load, first, xload))
                    for i in range(NT):
                        items.append((ear_load, first,
                                      lambda i=i, xts=xts, vts=vts:
                                      emit_vhat(xts, vts, i)))
                    for c in range(CT):
                        if bi == 0 and c == 0:
                            continue
                        # chunk c of batch bi: after cur batch stops reading
                        # chunk c (pair c+1 of batch bi-1), before (bi, c, 0)
                        if bi == 0:
                            ear = 0
                        elif c + 1 < CT:
                            ear = pidx[(bi - 1, c + 1, 0)]
                        else:
                            ear = pidx[(bi, 0, 0)]
                        ngr = 2 if xt_mode == "dma" else NT // TG
                        for g in range(ngr):
                            items.append((ear, pidx[(bi, c, 0)],
                                          lambda c=c, g=g, xts=xts, xtt=xtt,
                                          qts=qts, kts=kts:
                                          emit_chunk(xts, xtt, qts, kts, c,
                                                     groups=[g])))

                for bi in range(NB):
                    add_batch_items(bi)
                items.sort(key=lambda it: (it[0], it[1] if it[1] is not None
                                           else NPH))

                def flush(i, forced_deadline, budget=budget):
                    # emit all items whose deadline == forced_deadline, plus
                    # up to `budget` items whose earliest <= i
                    rest = []
                    n = 0
                    for it in items:
                        ear, dl, fn = it
                        if dl is not None and dl <= forced_deadline:
                            fn()
                        elif ear <= i and n < budget:
                            fn()
                            n += 1
                        else:
                            rest.append(it)
                    items[:] = rest

                # pipeline state
                po2_of, pts_of, pending = {}, {}, []

                def emit_s_exp(i, kc):
                    bi, c, qc = phases[i]
                    qts, kts, _ = qkv_of[bi]
                    ps = ps_s.tile([P, 2 * QB], FP32, tag="ps", name="ps")
                    # dup knobs re-emit identical ops (idempotent) to probe
                    # each engine's marginal slack
                    for rep_ in range(1 + dup_s):
                        for e in range(2):
                            nc.tensor.matmul(
                                ps[:, e * QB:(e + 1) * QB],
                                lhsT=kts[c][e * d:(e + 1) * d,
                                            kc * P:(kc + 1) * P],
                                rhs=qts[c][e * d:(e + 1) * d,
                                           qc * QB:(qc + 1) * QB],
                                start=True, stop=True)
                    if exp_split:
                        # per-head tiles: e0 exp on ACT, e1 Schraudolph on
                        # DVE -- separate tiles so the engines overlap
                        ptA = ptpool.tile([P, QB], BF16, tag="ptA",
                                          name="ptA")
                        ptB = ptpool.tile([P, QB], BF16, tag="ptB",
                                          name="ptB")
                        nc.scalar.activation(ptA[:], ps[:, 0:QB],
                                             AF.Exp, scale=scale)
                        nc.vector.tensor_scalar(ptB.bitcast(I16)[:],
                                                ps[:, QB:], A16, B16,
                                                op0=ALU.mult, op1=ALU.add)
                        pts_of[(i, kc)] = (ptA, ptB)
                        return
                    pt = ptpool.tile([P, 2 * QB], BF16, tag="pt", name="pt")
                    if kc in dve_kcs:
                        nc.vector.tensor_scalar(pt.bitcast(I16)[:], ps[:],
                                                A16, B16, op0=ALU.mult,
                                                op1=ALU.add)
                    else:
                        for rep_ in range(1 + dup_act):
                            nc.scalar.activation(pt[:], ps[:], AF.Exp,
                                                 scale=scale)
                    pts_of[(i, kc)] = pt

                def emit_pv(i, kc):
                    if stages < 3:
                        return
                    bi, c, qc = phases[i]
                    _, _, vts = qkv_of[bi]
                    h0 = 2 * c
                    if pv_nat:
                        # natural orientation: att slice is the stationary
                        # operand, Vhat [128, dO] per head is the moving one.
                        # out[q, c] accumulates over kc; Z lands in col d of
                        # each head's dO block (per-partition -> cheap DVE
                        # normalize, no PE transpose-back).
                        if i not in po2_of:
                            po2_of[i] = [
                                ps_o.tile([P, 4 * dO], FP32, tag="po",
                                          name=f"po{j}") for j in range(2)]
                        pt = pts_of.pop((i, kc))
                        for rep_ in range(1 + (dup_pv if kc == 0 else 0)):
                            for t in range(TB):
                                po = po2_of[i][t // 2]
                                off = (t % 2) * 2 * dO
                                for e in range(2):
                                    if isinstance(pt, tuple):
                                        lhsT = pt[e][:, t * P:(t + 1) * P]
                                    else:
                                        lhsT = pt[:, e * QB + t * P:
                                                  e * QB + (t + 1) * P]
                                    # start=True clears has_written for the
                                    # WHOLE bank: only the tile's first MM
                                    # may set it; the other groups' kc==0
                                    # MMs land on cleared bits (overwrite
                                    # mode) in PE program order.
                                    nc.tensor.matmul(
                                        po[:, off + e * dO:
                                           off + (e + 1) * dO],
                                        lhsT=lhsT,
                                        rhs=vts[kc][:, (h0 + e) * dO:
                                                    (h0 + e + 1) * dO],
                                        start=(kc == 0 and t % 2 == 0
                                               and e == 0),
                                        stop=(kc == NT - 1),
                                        skip_group_check=True)
                        return
                    if i not in po2_of:
                        po2_of[i] = [ps_o.tile([dO, QB], FP32, tag="po",
                                               name=f"po{e}") for e in range(2)]
                    pt = pts_of.pop((i, kc))
                    for rep_ in range(1 + (dup_pv if kc == 0 else 0)):
                        for e in range(2):
                            rhs = (pt[e][:, 0:QB] if isinstance(pt, tuple)
                                   else pt[:, e * QB:(e + 1) * QB])
                            nc.tensor.matmul(
                                po2_of[i][e][:],
                                lhsT=vts[kc][:,
                                             (h0 + e) * dO:(h0 + e + 1) * dO],
                                rhs=rhs,
                                start=(kc == 0), stop=(kc == NT - 1))

                def emit_drain(i):
                    if stages < 4 or stages < 3:
                        return
                    bi, c, qc = phases[i]
                    last_pair = (c == CT - 1)
                    h0 = 2 * c
                    outs = get_outs(bi)
                    po2 = po2_of.pop(i)
                    if pv_nat:
                        # po tile j holds subtiles t=2j,2j+1 as
                        # [e0(64) Z0 e1(64) Z1] x2; Z cols at 64+65k.
                        def finish_nat(j, po=None, bi=bi, qc=qc, h0=h0,
                                       last_pair=last_pair):
                            po = po2[j]
                            rz = rzpool.tile([P, 4], FP32, tag="rz",
                                             name="rz")
                            nc.vector.reciprocal(rz[:], po[:, d::dO])
                            for m in range(2):
                                t = 2 * j + m
                                qsub = qc * TB + t
                                dst = outs[qsub][:, h0 * d:(h0 + 2) * d]
                                if fin_tt:
                                    src = po[:, m * 2 * dO:(m + 1) * 2 * dO]
                                    s3 = src.rearrange("p (k x) -> p k x",
                                                       x=dO)
                                    r3 = rz[:, 2 * m:2 * m + 2]
                                    r3 = r3.rearrange("p (k x) -> p k x",
                                                      x=1)
                                    d3 = dst.rearrange("p (k x) -> p k x",
                                                      x=d)
                                    nc.vector.tensor_mul(
                                        d3[:], s3[:, :, 0:d],
                                        r3.broadcast_to([P, 2, d]))
                                else:
                                    for e in range(2):
                                        nc.vector.tensor_scalar_mul(
                                            outs[qsub][:, (h0 + e) * d:
                                                       (h0 + e + 1) * d],
                                            po[:, m * 2 * dO + e * dO:
                                               m * 2 * dO + e * dO + d],
                                            rz[:, 2 * m + e:2 * m + e + 1])
                                if last_pair:
                                    nc.sync.dma_start(
                                        out=out[batches[bi],
                                                qsub * P:(qsub + 1) * P, :],
                                        in_=outs[qsub][:])
                        pending.clear()
                        pending.append(lambda: finish_nat(0))
                        pending.append(lambda: finish_nat(1))
                        return
                    ots = []
                    for e in range(2):
                        ot = otpool.tile([dO, QB], FP32, tag=f"ot{e}",
                                         name=f"ot{e}")
                        nc.vector.tensor_copy(out=ot[:], in_=po2[e][:])
                        ots.append(ot)

                    def finish(e):
                        pn = ps_n.tile([P, TB * dO], FP32, tag="pn",
                                       name="pn")
                        for t in range(TB):
                            nc.tensor.transpose(
                                pn[:, t * dO:(t + 1) * dO],
                                ots[e][:, t * P:(t + 1) * P],
                                ident[0:dO, 0:dO])
                        rz = rzpool.tile([P, TB], FP32, tag="rz", name="rz")
                        nc.vector.reciprocal(rz[:], pn[:, d::dO])
                        for t in range(TB):
                            qsub = qc * TB + t
                            nc.vector.tensor_scalar_mul(
                                outs[qsub][:, (h0 + e) * d:(h0 + e + 1) * d],
                                pn[:, t * dO:t * dO + d],
                                rz[:, t:t + 1])
                        if last_pair and e == 1:
                            for t in range(TB):
                                qsub = qc * TB + t
                                nc.sync.dma_start(
                                    out=out[batches[bi],
                                            qsub * P:(qsub + 1) * P, :],
                                    in_=outs[qsub][:])
                    pending.clear()
                    pending.append(lambda: finish(0))
                    pending.append(lambda: finish(1))

                def flush_pending(all_=False):
                    while pending:
                        pending.pop(0)()
                        if not all_:
                            return

                # prologue: phase 0 prefetch
                flush(0, 0, budget=0)
                emit_s_exp(0, 0)
                for i in range(NPH):
                    bi, c, qc = phases[i]
                    dks = drain_kcs or (dkc, dkc + 3)
                    # pv_lag=2 keeps S(kc) AHEAD of PV(kc-2) in the PE FIFO:
                    # both wait on the same exp, but S is on the exp->ps-slot
                    # critical chain while PV is not, so issue S first.
                    for kc in range(1, NT):
                        emit_s_exp(i, kc)
                        if kc in dks:
                            flush_pending()
                        if kc - pv_lag >= 0 and kc < NT - 1:
                            emit_pv(i, kc - pv_lag)
                    if i + 1 < NPH:
                        flush(i, i + 1)
                        emit_s_exp(i + 1, 0)
                    for kc in range(NT - 1 - pv_lag, NT):
                        if kc >= 0:
                            emit_pv(i, kc)
                    emit_drain(i)
                    # end of batch: flush the last drain, drop out-tile refs
                    if stages >= 4 and (i + 1 == NPH or phases[i + 1][0] != bi):
                        flush_pending(all_=True)
                        outs_of.pop(bi)
                flush(NPH, NPH, budget=99)

            if loop_reps is None:
                emit_program([bb for _ in range(reps) for bb in range(BPC)])
            else:
                with tc.For_i(0, loop_reps, 1):
                    emit_program(list(range(BPC)))
    nc.compile()
    return nc


_built = {}

# chosen production config (see sweep logs): PV-natural, exp split 5 ACT /
# 3 DVE-Schraudolph chunks, S-before-PV queue order
BEST = dict(dve_kcs=(1, 4, 6), pv_lag=2, ptb=6)


def _get_nc(BPC):
    if BPC not in _built:
        _built[BPC] = build(BPC=BPC, **BEST)
    return _built[BPC]


def prep_inputs(x, wq, bq, wk, bk, wv, bv, wo, bo):
    x = np.ascontiguousarray(np.asarray(x, dtype=np.float32))
    wq, bq, wk, bk, wv, bv, wo, bo = (
        np.asarray(t, dtype=np.float32)
        for t in (wq, bq, wk, bk, wv, bv, wo, bo))
    xb = x.astype(ml_dtypes.bfloat16)
    veff = (wv * wo).astype(ml_dtypes.bfloat16)
    beff = (bv * wo + bo).astype(ml_dtypes.bfloat16)
    return xb, wq, bq, wk, bk, veff, beff


def kernel(x, wq, bq, wk, bk, wv, bv, wo, bo):
    xb, wq, bq, wk, bk, veff, beff = prep_inputs(
        x, wq, bq, wk, bk, wv, bv, wo, bo)
    Bx = xb.shape[0]
    BPC = Bx // N_CORES
    assert BPC * N_CORES == Bx, (Bx, N_CORES)
    nc = _get_nc(BPC)
    in_maps = []
    for i in range(N_CORES):
        in_maps.append({
            "x": xb[i * BPC:(i + 1) * BPC],
            "wq": wq, "bq": bq, "wk": wk, "bk": bk,
            "veff": veff, "beff": beff,
        })
    res = run_bass_kernel_spmd(nc, in_maps, list(range(N_CORES)))
    return np.concatenate([r["out"] for r in res.results], axis=0)


if __name__ == "__main__":
    rng = np.random.default_rng(1)
    inputs = {
        "x": rng.standard_normal((B, N, F), dtype=np.float32),
        "wq": rng.standard_normal((F,), dtype=np.float32),
        "bq": np.zeros(F, np.float32),
        "wk": rng.standard_normal((F,), dtype=np.float32),
        "bk": np.zeros(F, np.float32),
        "wv": rng.standard_normal((F,), dtype=np.float32),
        "bv": np.zeros(F, np.float32),
        "wo": rng.standard_normal((F,), dtype=np.float32),
        "bo": np.zeros(F, np.float32),
    }
    o = kernel(**inputs)
    print("out", o.shape, o.dtype)



# revision 18
# speedup vs baseline: 1.2122x; 1.2006x over previous
"""Trainium2 Bass kernel: MultiHeadDepthwiseSelfAttention.

Full inputs -> data-parallel over batch across 8 NeuronCores -> full output.

Math (reference):
    q = x*wq + bq; k = x*wk + bk; v = x*wv + bv      (per-channel depthwise)
    att = softmax(q @ k^T / sqrt(F))  per head        (H=12, d=64)
    out = (att @ v) * wo + bo

Kernel strategy (per core, 2 batches):
  - Host folds the output projection into v:  veff = wv*wo, beff = bv*wo + bo.
    Then out = (att @ (x*veff + beff_aug)) / Z  where the +bo fold works because
    appending bo[c] to every V row adds bo[c]*Z[q] to the unnormalized output.
  - On chip, work in the transposed orientation S^T[k,q] so softmax's
    normalization sum (over k) is computed by the PV matmul itself via an
    extra ones-column appended per head to V ("Vhat": H*(d+1) columns).
    exp() needs no max-subtraction: logits are O(1) by construction.
  - x ships as bf16; x^T tiles come from DMA xbar transposes (no PE time).
    Q^T/K^T are per-partition scale+bias (DVE) in bf16.
  - Matmuls run in bf16 (1 col/cycle); the two heads of a pair row-pack the
    PE array (K=64 each at row groups 0/64) and run concurrently.
  - exp is split: most k-chunks on ScalarE (ACT exp -> bf16), `dve_kcs`
    chunks on VectorE via the Schraudolph trick: bf16_bits(int16(A*s + B))
    approximates exp (~3% sawtooth; numerator+denominator partially cancel).
    The exp->S(kc+2) PSUM-slot chain is the critical path; dve_kcs picks
    which links run on DVE so ACT and DVE alternate along the chain.
  - PV runs in natural orientation (pv_nat): exp(S^T) slices [128k, 128q]
    are the stationary operand (FWL), Vhat [128, 65] per head the moving
    one; out [q, 65] accumulates over kc in PSUM. The Z ones-column lands
    per-partition, so softmax finishes with a cheap reciprocal + broadcast
    TT mul -- no PE transpose-back, no O^T drain. PSUM start=True clears
    has_written BANK-wide: only the first MM per po bank sets it.
  - pv_lag=2 issues S(kc) ahead of PV(kc-2) in the PE FIFO so the
    exp-chain link cost is exp+S, not exp+PV+S.
"""

import math
import os
import sys

for _p in ("/opt/trn_rl_repo", "/root/.axon_site/_ro/trn_rl_repo"):
    if os.path.isdir(_p) and _p not in sys.path:
        sys.path.insert(0, _p)

import numpy as np
import ml_dtypes

import concourse.bacc as bacc
import concourse.mybir as mybir
from concourse.tile import TileContext
from concourse.masks import make_identity
from concourse.bass_utils import run_bass_kernel_spmd

FP32 = mybir.dt.float32
BF16 = mybir.dt.bfloat16
I16 = mybir.dt.int16
AF = mybir.ActivationFunctionType
ALU = mybir.AluOpType

P = 128
N_CORES = 8
B, N, F, H = 16, 1024, 768, 12
LOG2E = 1.4426950408889634


def build(BPC=2, N=N, F=F, H=H, reps=1, loop_reps=None, stages=4,
          ptb=4, otb=1, budget=1, dkc=1, dve_kcs=(), C16=0.0,
          xt_mode="dma", ps_sb=2, ps_ob=3, ps_nb=1, gp_vhat=False,
          drain_kcs=None, exp_split=0, gp_qk=False, pv_nat=True, fin_tt=1,
          pv_lag=1, exp2=0, fin_pair=0,
          dup_s=0, dup_pv=0, dup_act=0, dup_dve=0, dup_xt=0):
    d = F // H            # head dim (64)
    dO = d + 1            # V columns per head incl. ones column
    NT = N // P           # n-tiles (= k-chunks)
    CT = F // P           # channel chunks (== head pairs)
    QB = min(512, N)      # q block (moving-dim) size
    QC = N // QB          # q blocks
    TB = QB // P          # natural q-subtiles per q block
    scale = 1.0 / math.sqrt(F)
    A16 = float((1 << 7) * LOG2E * scale)
    B16 = float(127 * (1 << 7)) - float(C16)
    assert P % d == 0 and CT == H // 2

    if pv_nat:
        ps_ob = 4  # two [P, 2*dO] po tiles per phase, double-buffered
    nc = bacc.Bacc("TRN2", target_bir_lowering=False, debug=False,
                   num_devices=N_CORES)
    x = nc.declare_dram_parameter("x", [BPC, N, F], BF16, isOutput=False)
    wq = nc.declare_dram_parameter("wq", [F], FP32, isOutput=False)
    bq = nc.declare_dram_parameter("bq", [F], FP32, isOutput=False)
    wk = nc.declare_dram_parameter("wk", [F], FP32, isOutput=False)
    bk = nc.declare_dram_parameter("bk", [F], FP32, isOutput=False)
    veff = nc.declare_dram_parameter("veff", [F], BF16, isOutput=False)
    beff = nc.declare_dram_parameter("beff", [F], BF16, isOutput=False)
    out = nc.declare_dram_parameter("out", [BPC, N, F], FP32, isOutput=True)

    with TileContext(nc) as tc:
        with (
            tc.tile_pool(name="const", bufs=1) as cpool,
            tc.tile_pool(name="xp", bufs=2) as xpool,
            tc.tile_pool(name="xtp", bufs=2) as xtpool,
            tc.tile_pool(name="qp", bufs=1) as qpool,
            tc.tile_pool(name="kp", bufs=1) as kpool,
            tc.tile_pool(name="vp", bufs=2) as vpool,
            tc.tile_pool(name="op", bufs=1) as opool,
            tc.tile_pool(name="ptp", bufs=ptb) as ptpool,
            tc.tile_pool(name="otp", bufs=otb) as otpool,
            tc.tile_pool(name="rzp", bufs=2) as rzpool,
            tc.tile_pool(name="ps_s", bufs=ps_sb, space="PSUM") as ps_s,
            tc.tile_pool(name="ps_o", bufs=ps_ob, space="PSUM") as ps_o,
            tc.tile_pool(name="ps_n", bufs=ps_nb, space="PSUM") as ps_n,
        ):
            ident = cpool.tile([P, P], FP32)
            make_identity(nc, ident[:])
            ident_b = cpool.tile([P, P], BF16)
            nc.vector.tensor_copy(out=ident_b[:], in_=ident[:])
            wq_c = cpool.tile([P, CT], FP32)
            bq_c = cpool.tile([P, CT], FP32)
            wk_c = cpool.tile([P, CT], FP32)
            bk_c = cpool.tile([P, CT], FP32)
            veff_b = cpool.tile([P, F], BF16)
            beff_b = cpool.tile([P, F], BF16)

            def emit_bcasts():
                nc.sync.dma_start(out=veff_b[:],
                                  in_=veff[None, :].broadcast_to([P, F]))
                nc.sync.dma_start(out=beff_b[:],
                                  in_=beff[None, :].broadcast_to([P, F]))

            def emit_weight_loads():
                # contiguous row loads + PE transposes into per-partition cols
                specs = ((wq_c, wq), (bq_c, bq), (wk_c, wk), (bk_c, bk))
                rows = []
                for wi, (dst, src) in enumerate(specs):
                    row = cpool.tile([1, F], FP32, tag=f"wrow{wi}",
                                     name=f"wrow{wi}")
                    nc.sync.dma_start(out=row[:], in_=src[None, :])
                    rows.append(row)
                if pv_nat:
                    pw = ps_o.tile([P, 4 * CT], FP32, tag="po", name="pw")
                else:
                    pw = ps_n.tile([P, 4 * CT], FP32, tag="pn", name="pw")
                for wi, (dst, src) in enumerate(specs):
                    for c in range(CT):
                        nc.tensor.transpose(
                            pw[:, wi * CT + c:wi * CT + c + 1],
                            rows[wi][0:1, c * P:(c + 1) * P],
                            ident[0:1, 0:1])
                for wi, (dst, src) in enumerate(specs):
                    nc.vector.tensor_copy(out=dst[:],
                                          in_=pw[:, wi * CT:(wi + 1) * CT])

            def alloc_qkv(_b):
                qts = [qpool.tile([P, N], BF16, tag=f"qt{c}", name=f"qt{c}")
                       for c in range(CT)]
                kts = [kpool.tile([P, N], BF16, tag=f"kt{c}", name=f"kt{c}")
                       for c in range(CT)]
                vts = [vpool.tile([P, H * dO], BF16, tag=f"vt{i}",
                                  name=f"vt{i}") for i in range(NT)]
                return qts, kts, vts

            TG = 4  # PE-mode transposes per psum group

            def emit_chunk(xts, xtt, qts, kts, c, groups=None):
                # Q^T/K^T rows for head pair c (DVE scale+bias), x^T either
                # DMA-transposed (xtt) or PE-transposed from natural xts
                if xt_mode == "dma":
                    veng = nc.gpsimd if gp_qk else nc.vector
                    for g in (range(2) if groups is None else groups):
                        sl = slice(g * (N // 2), (g + 1) * (N // 2))
                        veng.tensor_scalar(
                            qts[c][:, sl], xtt[c][:, sl],
                            wq_c[:, c:c + 1], bq_c[:, c:c + 1],
                            op0=ALU.mult, op1=ALU.add)
                        veng.tensor_scalar(
                            kts[c][:, sl], xtt[c][:, sl],
                            wk_c[:, c:c + 1], bk_c[:, c:c + 1],
                            op0=ALU.mult, op1=ALU.add)
                    return
                for g in (range(NT // TG) if groups is None else groups):
                    pst = ps_n.tile([P, TG * P], BF16, tag="pnb", name="pst")
                    for j in range(TG):
                        i = g * TG + j
                        nc.tensor.transpose(pst[:, j * P:(j + 1) * P],
                                            xts[i][:, c * P:(c + 1) * P],
                                            ident_b[:])
                    sl = slice(g * TG * P, (g + 1) * TG * P)
                    nc.vector.tensor_scalar(qts[c][:, sl], pst[:],
                                            wq_c[:, c:c + 1], bq_c[:, c:c + 1],
                                            op0=ALU.mult, op1=ALU.add)
                    nc.vector.tensor_scalar(kts[c][:, sl], pst[:],
                                            wk_c[:, c:c + 1], bk_c[:, c:c + 1],
                                            op0=ALU.mult, op1=ALU.add)

            def emit_vhat(xts, vts, i):
                v3 = vts[i].rearrange("p (h e) -> p h e", e=dO)
                x3 = xts[i].rearrange("p (h e) -> p h e", e=d)
                w3 = veff_b.rearrange("p (h e) -> p h e", e=d)
                b3 = beff_b.rearrange("p (h e) -> p h e", e=d)
                eng = nc.gpsimd if gp_vhat else nc.vector
                for rep_ in range(1 + dup_dve):
                    nc.vector.tensor_scalar(vts[i][:, d::dO], veff_b[:, 0:H],
                                            0.0, 1.0, op0=ALU.mult,
                                            op1=ALU.add)
                    eng.tensor_mul(v3[:, :, 0:d], x3[:], w3[:])
                    eng.tensor_add(v3[:, :, 0:d], v3[:, :, 0:d], b3[:])

            def emit_program(batches):
                NB = len(batches)
                phases = [(bi, c, qc) for bi in range(NB)
                          for c in range(CT) for qc in range(QC)]
                NPH = len(phases)
                pidx = {ph: i for i, ph in enumerate(phases)}

                # per-batch tile state, allocated lazily
                xts_of, xtt_of, qkv_of, outs_of = {}, {}, {}, {}

                def get_outs(bi):
                    if bi not in outs_of:
                        if pv_nat and fin_pair:
                            # qsub pair tiles: qsub q lives at columns
                            # (q%2)*F of pair tile q//2
                            outs_of[bi] = {
                                p: opool.tile([P, 2 * F], FP32,
                                              tag=f"op{p}", name=f"op{p}")
                                for p in range(NT // 2)}
                        else:
                            outs_of[bi] = {
                                i: opool.tile([P, F], FP32, tag=f"on{i}",
                                              name=f"on{i}")
                                for i in range(NT)}
                    return outs_of[bi]

                # work items: (earliest, deadline, fn); deadline=None -> any
                items = []

                def add_batch_items(bi):
                    xts_of[bi] = [xpool.tile([P, F], BF16, tag=f"xt{i}",
                                             name=f"xt{i}") for i in range(NT)]
                    xtt_of[bi] = [xtpool.tile([P, N], BF16, tag=f"xtt{c}",
                                              name=f"xtt{c}")
                                  for c in range(CT)]
                    qkv_of[bi] = alloc_qkv(bi)
                    xts, xtt = xts_of[bi], xtt_of[bi]
                    qts, kts, vts = qkv_of[bi]
                    first = pidx[(bi, 0, 0)]
                    if bi == 0:
                        ear_load = 0
                    else:
                        ear_load = pidx[(bi - 1, 0, 0)]

                    def xload(bi=bi, xts=xts, xtt=xtt):
                        # chunk-0 x^T first (unblocks pair-0 preproc fast)
                        if xt_mode == "dma":
                            nc.sync.dma_start_transpose(
                                xtt[0][:], x[batches[bi], :, 0:P])
                        for i in range(NT):
                            nc.sync.dma_start(
                                out=xts[i][:],
                                in_=x[batches[bi], i * P:(i + 1) * P, :])
                        if xt_mode == "dma":
                            for rep_ in range(1 + dup_xt):
                                for c in range(1, CT):
                                    nc.sync.dma_start_transpose(
                                        xtt[c][:],
                                        x[batches[bi], :, c * P:(c + 1) * P])
                    if bi == 0:
                        def first_loads():
                            emit_weight_loads()
                            emit_bcasts()
                            xload()
                            emit_chunk(xts, xtt, qts, kts, 0)
                        items.append((0, 0, first_loads))
                    else:
                        items.append((ear_load, first, xload))
                    for i in range(NT):
                        items.append((ear_load, first,
                                      lambda i=i, xts=xts, vts=vts:
                                      emit_vhat(xts, vts, i)))
                    for c in range(CT):
                        if bi == 0 and c == 0:
                            continue
                        # chunk c of batch bi: after cur batch stops reading
                        # chunk c (pair c+1 of batch bi-1), before (bi, c, 0)
                        if bi == 0:
                            ear = 0
                        elif c + 1 < CT:
                            ear = pidx[(bi - 1, c + 1, 0)]
                        else:
                            ear = pidx[(bi, 0, 0)]
                        ngr = 2 if xt_mode == "dma" else NT // TG
                        for g in range(ngr):
                            items.append((ear, pidx[(bi, c, 0)],
                                          lambda c=c, g=g, xts=xts, xtt=xtt,
                                          qts=qts, kts=kts:
                                          emit_chunk(xts, xtt, qts, kts, c,
                                                     groups=[g])))

                for bi in range(NB):
                    add_batch_items(bi)
                items.sort(key=lambda it: (it[0], it[1] if it[1] is not None
                                           else NPH))

                def flush(i, forced_deadline, budget=budget):
                    # emit all items whose deadline == forced_deadline, plus
                    # up to `budget` items whose earliest <= i
                    rest = []
                    n = 0
                    for it in items:
                        ear, dl, fn = it
                        if dl is not None and dl <= forced_deadline:
                            fn()
                        elif ear <= i and n < budget:
                            fn()
                            n += 1
                        else:
                            rest.append(it)
                    items[:] = rest

                # pipeline state
                po2_of, pts_of, pending = {}, {}, []

                def emit_s_exp(i, kc):
                    bi, c, qc = phases[i]
                    qts, kts, _ = qkv_of[bi]
                    ps = ps_s.tile([P, 2 * QB], FP32, tag="ps", name="ps")
                    # dup knobs re-emit identical ops (idempotent) to probe
                    # each engine's marginal slack
                    for rep_ in range(1 + dup_s):
                        for e in range(2):
                            nc.tensor.matmul(
                                ps[:, e * QB:(e + 1) * QB],
                                lhsT=kts[c][e * d:(e + 1) * d,
                                            kc * P:(kc + 1) * P],
                                rhs=qts[c][e * d:(e + 1) * d,
                                           qc * QB:(qc + 1) * QB],
                                start=True, stop=True)
                    if exp_split:
                        # per-head tiles: e0 exp on ACT, e1 Schraudolph on
                        # DVE -- separate tiles so the engines overlap
                        ptA = ptpool.tile([P, QB], BF16, tag="ptA",
                                          name="ptA")
                        ptB = ptpool.tile([P, QB], BF16, tag="ptB",
                                          name="ptB")
                        nc.scalar.activation(ptA[:], ps[:, 0:QB],
                                             AF.Exp, scale=scale)
                        nc.vector.tensor_scalar(ptB.bitcast(I16)[:],
                                                ps[:, QB:], A16, B16,
                                                op0=ALU.mult, op1=ALU.add)
                        pts_of[(i, kc)] = (ptA, ptB)
                        return
                    pt = ptpool.tile([P, 2 * QB], BF16, tag="pt", name="pt")
                    if kc in dve_kcs:
                        if exp2 >= 2:
                            # per-head halves: PSUM-bank-aligned so the
                            # region-dep lets S(kc+2) of head e start as
                            # soon as this chunk's head-e half is read
                            for e in range(2):
                                sl = slice(e * QB, (e + 1) * QB)
                                nc.vector.tensor_scalar(
                                    pt.bitcast(I16)[:, sl], ps[:, sl],
                                    A16, B16, op0=ALU.mult, op1=ALU.add)
                        else:
                            nc.vector.tensor_scalar(pt.bitcast(I16)[:],
                                                    ps[:], A16, B16,
                                                    op0=ALU.mult,
                                                    op1=ALU.add)
                    elif exp2:
                        for rep_ in range(1 + dup_act):
                            for e in range(2):
                                sl = slice(e * QB, (e + 1) * QB)
                                nc.scalar.activation(pt[:, sl], ps[:, sl],
                                                     AF.Exp, scale=scale)
                    else:
                        for rep_ in range(1 + dup_act):
                            nc.scalar.activation(pt[:], ps[:], AF.Exp,
                                                 scale=scale)
                    pts_of[(i, kc)] = pt

                def emit_pv(i, kc):
                    if stages < 3:
                        return
                    bi, c, qc = phases[i]
                    _, _, vts = qkv_of[bi]
                    h0 = 2 * c
                    if pv_nat:
                        # natural orientation: att slice is the stationary
                        # operand, Vhat [128, dO] per head is the moving one.
                        # out[q, c] accumulates over kc; Z lands in col d of
                        # each head's dO block (per-partition -> cheap DVE
                        # normalize, no PE transpose-back).
                        if i not in po2_of:
                            po2_of[i] = [
                                ps_o.tile([P, 4 * dO], FP32, tag="po",
                                          name=f"po{j}") for j in range(2)]
                        pt = pts_of.pop((i, kc))
                        for rep_ in range(1 + (dup_pv if kc == 0 else 0)):
                            for t in range(TB):
                                po = po2_of[i][t // 2]
                                off = (t % 2) * 2 * dO
                                for e in range(2):
                                    if isinstance(pt, tuple):
                                        lhsT = pt[e][:, t * P:(t + 1) * P]
                                    else:
                                        lhsT = pt[:, e * QB + t * P:
                                                  e * QB + (t + 1) * P]
                                    # start=True clears has_written for the
                                    # WHOLE bank: only the tile's first MM
                                    # may set it; the other groups' kc==0
                                    # MMs land on cleared bits (overwrite
                                    # mode) in PE program order.
                                    nc.tensor.matmul(
                                        po[:, off + e * dO:
                                           off + (e + 1) * dO],
                                        lhsT=lhsT,
                                        rhs=vts[kc][:, (h0 + e) * dO:
                                                    (h0 + e + 1) * dO],
                                        start=(kc == 0 and t % 2 == 0
                                               and e == 0),
                                        stop=(kc == NT - 1),
                                        skip_group_check=True)
                        return
                    if i not in po2_of:
                        po2_of[i] = [ps_o.tile([dO, QB], FP32, tag="po",
                                               name=f"po{e}") for e in range(2)]
                    pt = pts_of.pop((i, kc))
                    for rep_ in range(1 + (dup_pv if kc == 0 else 0)):
                        for e in range(2):
                            rhs = (pt[e][:, 0:QB] if isinstance(pt, tuple)
                                   else pt[:, e * QB:(e + 1) * QB])
                            nc.tensor.matmul(
                                po2_of[i][e][:],
                                lhsT=vts[kc][:,
                                             (h0 + e) * dO:(h0 + e + 1) * dO],
                                rhs=rhs,
                                start=(kc == 0), stop=(kc == NT - 1))

                def emit_drain(i):
                    if stages < 4 or stages < 3:
                        return
                    bi, c, qc = phases[i]
                    last_pair = (c == CT - 1)
                    h0 = 2 * c
                    outs = get_outs(bi)
                    po2 = po2_of.pop(i)
                    if pv_nat:
                        # po tile j holds subtiles t=2j,2j+1 as
                        # [e0(64) Z0 e1(64) Z1] x2; Z cols at 64+65k.
                        def finish_nat(j, po=None, bi=bi, qc=qc, h0=h0,
                                       last_pair=last_pair):
                            po = po2[j]
                            rz = rzpool.tile([P, 4], FP32, tag="rz",
                                             name="rz")
                            nc.vector.reciprocal(rz[:], po[:, d::dO])
                            if fin_pair:
                                # one TT for both subtiles of this po tile
                                pair = outs[qc * 2 + j]
                                d4 = pair.rearrange("p (m f) -> p m f", f=F)
                                d4 = d4[:, :, h0 * d:(h0 + 2) * d]
                                d4 = d4.rearrange("p m (e x) -> p m e x",
                                                  x=d)
                                s4 = po.rearrange("p (m k x) -> p m k x",
                                                  k=2, x=dO)
                                r4 = rz.rearrange("p (m e x) -> p m e x",
                                                  m=2, x=1)
                                nc.vector.tensor_mul(
                                    d4[:], s4[:, :, :, 0:d],
                                    r4.broadcast_to([P, 2, 2, d]))
                                if last_pair:
                                    for m in range(2):
                                        qsub = qc * TB + 2 * j + m
                                        nc.sync.dma_start(
                                            out=out[batches[bi],
                                                    qsub * P:(qsub + 1) * P,
                                                    :],
                                            in_=pair[:, m * F:(m + 1) * F])
                                return
                            for m in range(2):
                                t = 2 * j + m
                                qsub = qc * TB + t
                                dst = outs[qsub][:, h0 * d:(h0 + 2) * d]
                                if fin_tt:
                                    src = po[:, m * 2 * dO:(m + 1) * 2 * dO]
                                    s3 = src.rearrange("p (k x) -> p k x",
                                                       x=dO)
                                    r3 = rz[:, 2 * m:2 * m + 2]
                                    r3 = r3.rearrange("p (k x) -> p k x",
                                                      x=1)
                                    d3 = dst.rearrange("p (k x) -> p k x",
                                                      x=d)
                                    nc.vector.tensor_mul(
                                        d3[:], s3[:, :, 0:d],
                                        r3.broadcast_to([P, 2, d]))
                                else:
                                    for e in range(2):
                                        nc.vector.tensor_scalar_mul(
                                            outs[qsub][:, (h0 + e) * d:
                                                       (h0 + e + 1) * d],
                                            po[:, m * 2 * dO + e * dO:
                                               m * 2 * dO + e * dO + d],
                                            rz[:, 2 * m + e:2 * m + e + 1])
                                if last_pair:
                                    nc.sync.dma_start(
                                        out=out[batches[bi],
                                                qsub * P:(qsub + 1) * P, :],
                                        in_=outs[qsub][:])
                        pending.clear()
                        pending.append(lambda: finish_nat(0))
                        pending.append(lambda: finish_nat(1))
                        return
                    ots = []
                    for e in range(2):
                        ot = otpool.tile([dO, QB], FP32, tag=f"ot{e}",
                                         name=f"ot{e}")
                        nc.vector.tensor_copy(out=ot[:], in_=po2[e][:])
                        ots.append(ot)

                    def finish(e):
                        pn = ps_n.tile([P, TB * dO], FP32, tag="pn",
                                       name="pn")
                        for t in range(TB):
                            nc.tensor.transpose(
                                pn[:, t * dO:(t + 1) * dO],
                                ots[e][:, t * P:(t + 1) * P],
                                ident[0:dO, 0:dO])
                        rz = rzpool.tile([P, TB], FP32, tag="rz", name="rz")
                        nc.vector.reciprocal(rz[:], pn[:, d::dO])
                        for t in range(TB):
                            qsub = qc * TB + t
                            nc.vector.tensor_scalar_mul(
                                outs[qsub][:, (h0 + e) * d:(h0 + e + 1) * d],
                                pn[:, t * dO:t * dO + d],
                                rz[:, t:t + 1])
                        if last_pair and e == 1:
                            for t in range(TB):
                                qsub = qc * TB + t
                                nc.sync.dma_start(
                                    out=out[batches[bi],
                                            qsub * P:(qsub + 1) * P, :],
                                    in_=outs[qsub][:])
                    pending.clear()
                    pending.append(lambda: finish(0))
                    pending.append(lambda: finish(1))

                def flush_pending(all_=False):
                    while pending:
                        pending.pop(0)()
                        if not all_:
                            return

                # prologue: phase 0 prefetch
                flush(0, 0, budget=0)
                emit_s_exp(0, 0)
                for i in range(NPH):
                    bi, c, qc = phases[i]
                    dks = drain_kcs or (dkc, dkc + 3)
                    # pv_lag=2 keeps S(kc) AHEAD of PV(kc-2) in the PE FIFO:
                    # both wait on the same exp, but S is on the exp->ps-slot
                    # critical chain while PV is not, so issue S first.
                    for kc in range(1, NT):
                        emit_s_exp(i, kc)
                        if kc in dks:
                            flush_pending()
                        if kc - pv_lag >= 0 and kc < NT - 1:
                            emit_pv(i, kc - pv_lag)
                    if i + 1 < NPH:
                        flush(i, i + 1)
                        emit_s_exp(i + 1, 0)
                    for kc in range(NT - 1 - pv_lag, NT):
                        if kc >= 0:
                            emit_pv(i, kc)
                    emit_drain(i)
                    # end of batch: flush the last drain, drop out-tile refs
                    if stages >= 4 and (i + 1 == NPH or phases[i + 1][0] != bi):
                        flush_pending(all_=True)
                        outs_of.pop(bi)
                flush(NPH, NPH, budget=99)

            if loop_reps is None:
                emit_program([bb for _ in range(reps) for bb in range(BPC)])
            else:
                with tc.For_i(0, loop_reps, 1):
                    emit_program(list(range(BPC)))
    nc.compile()
    return nc


_built = {}

# chosen production config (see sweep logs): PV-natural, exp split 5 ACT /
# 3 DVE-Schraudolph chunks, S-before-PV queue order
BEST = dict(dve_kcs=(1, 4, 6), pv_lag=2, ptb=6)


def _get_nc(BPC):
    if BPC not in _built:
        _built[BPC] = build(BPC=BPC, **BEST)
    return _built[BPC]


def prep_inputs(x, wq, bq, wk, bk, wv, bv, wo, bo):
    x = np.ascontiguousarray(np.asarray(x, dtype=np.float32))
    wq, bq, wk, bk, wv, bv, wo, bo = (
        np.asarray(t, dtype=np.float32)
        for t in (wq, bq, wk, bk, wv, bv, wo, bo))
    xb = x.astype(ml_dtypes.bfloat16)
    veff = (wv * wo).astype(ml_dtypes.bfloat16)
    beff = (bv * wo + bo).astype(ml_dtypes.bfloat16)
    return xb, wq, bq, wk, bk, veff, beff


def kernel(x, wq, bq, wk, bk, wv, bv, wo, bo):
    xb, wq, bq, wk, bk, veff, beff = prep_inputs(
        x, wq, bq, wk, bk, wv, bv, wo, bo)
    Bx = xb.shape[0]
    BPC = Bx // N_CORES
    assert BPC * N_CORES == Bx, (Bx, N_CORES)
    nc = _get_nc(BPC)
    in_maps = []
    for i in range(N_CORES):
        in_maps.append({
            "x": xb[i * BPC:(i + 1) * BPC],
            "wq": wq, "bq": bq, "wk": wk, "bk": bk,
            "veff": veff, "beff": beff,
        })
    res = run_bass_kernel_spmd(nc, in_maps, list(range(N_CORES)))
    return np.concatenate([r["out"] for r in res.results], axis=0)


if __name__ == "__main__":
    rng = np.random.default_rng(1)
    inputs = {
        "x": rng.standard_normal((B, N, F), dtype=np.float32),
        "wq": rng.standard_normal((F,), dtype=np.float32),
        "bq": np.zeros(F, np.float32),
        "wk": rng.standard_normal((F,), dtype=np.float32),
        "bk": np.zeros(F, np.float32),
        "wv": rng.standard_normal((F,), dtype=np.float32),
        "bv": np.zeros(F, np.float32),
        "wo": rng.standard_normal((F,), dtype=np.float32),
        "bo": np.zeros(F, np.float32),
    }
    o = kernel(**inputs)
    print("out", o.shape, o.dtype)

